# revision 31
# baseline (speedup 1.0000x reference)
"""Trainium2 Bass kernel for nn_CNNCrossPatchBackbone (sparse cross-patch attention).

Strategy: 8 cores = 4 batches x {ctx self-attention, tgt cross-attention}.
The two MHAs of one batch share only the (read-only) context tokens, so the
work is fully task-parallel: no collectives. Each core runs an identical-shape
problem: 1024 q-tokens x 1024 kv-tokens, 16 heads of dim 64, D=1024.

All matmul operands are bf16 (same 1 cycle/row PE stream rate as float32r,
half the DMA/LDWEIGHTS bytes, 2x DVE throughput); accumulation is fp32 in
PSUM. The softmax denominator reciprocal path stays fp32.

Host side: stable argsort of is_context, token gather, rope-cache gather by
clipped integer coords, pair-split permutation of the D axis, weight
transposes, 1/sqrt(hd)=2^-3 folded into q-projection weights+bias, x
pre-transposed to d-major (no device transposes), V bias folded into the
output-projection bias (b' = b_o + W_o b_v), Q weights packed chain-major so
the Q projection starts while its weights are still streaming in.

Device side per core:
  1. rope rotation on DVE directly on the DMA'd x^T tiles (bf16).
  2. QKV projections: Q^T tiles [2 heads' dims, tok]; K^T written into
     persistent zero-padded per-head tiles KT_pad[h] (64 data rows at the
     head's partition offset, rest zero) so S contractions run K=128
     (HAM needs full array activity to unthrottle the PE clock); V in
     natural [tok, dout] layout with a ones-column per head.
  3. Per (head, q-half) block, software-pipelined: S^T = KT_pad[h]^T Q,
     exp on ACT (max-subtraction skipped: scores ~N(0,1)) to bf16, O^T
     accumulated over keys in PSUM; the ones-column gives the softmax
     denominator in row 64; the two q-halves of a head are reciprocal'd
     together and partition-broadcast on the (idle) GpSimd engine, then
     multiplied in on DVE.
  4. Output projection split into two 4-matmul half-chains: heads 0-7
     (ready after half the attention blocks) are projected inside the
     late attention slots, filling PE bubbles of the exp-paced pipeline;
     heads 8-15 after the last block. Bias b' added during the first
     half's PSUM->SBUF copy on DVE. DMA out per 128-token tile.
"""

import sys

sys.path.insert(0, "/opt/trn_rl_repo")

import ml_dtypes
import numpy as np

import concourse.bass as bass  # noqa: F401
import concourse.tile as tile
from concourse import bacc, mybir
from concourse.bass_utils import run_bass_kernel_spmd

B, K, D, H = 4, 2048, 1024, 16
NCTX = K // 2
NTOK = 1024  # tokens per side after the ctx/tgt split
HD = D // H  # 64
IMAGE_SIZE = 224.0
MAX_POS = 1024
P = 128
DT = D // P  # 8 d-tiles
TT = NTOK // P  # 8 token-tiles
F32 = mybir.dt.float32
F32R = mybir.dt.float32r
BF16 = mybir.dt.bfloat16
NPBF16 = ml_dtypes.bfloat16

# gpsimd partition-broadcast for the softmax denominators (else a K=2
# PE matmul). Measured: the broadcast's gpsimd library slows EVERY matmul
# by ~34ns (305us vs 292us total) — keep False.
USE_GPS_BCAST = False
# interleave the first half of the output projection (heads 0-7, ready
# after half the attention blocks) into late attention slots; the half1
# chains share one PSUM bank with the rb broadcast tiles (same shape+tag).
# Measured: 296.4us vs 294.1us without — the ring coupling costs more than
# the bubbles it fills. Keep False.
USE_SPLIT_Y = False
# fine-grained S/AV interleave: emit the previous block's AV accumulation
# pairs between this block's S pairs. Measured: 299.6us vs 294.1us blocked
# (operand-stream thrash) — keep False.
FINE_INTERLEAVE = False
# run ~40 junk matmuls during the DMA lead-in so the PE p-state ramps to
# 2.4GHz before the Q projection (first ~25 real matmuls otherwise run at
# the 1.2GHz mid p-state: 634ns vs 378ns each)
PE_WARMUP = 64

# pair-split permutation: [x-evens, x-odds, y-evens, y-odds]
PERM = np.concatenate(
    [
        np.arange(0, 512, 2),
        np.arange(1, 512, 2),
        np.arange(512, 1024, 2),
        np.arange(513, 1024, 2),
    ]
)


def build_nc():
    nc = bacc.Bacc("TRN2", target_bir_lowering=False, debug=False, num_devices=8)

    xq_ext = nc.dram_tensor("xqT", [DT, P, NTOK], BF16, kind="ExternalInput")
    xkv_ext = nc.dram_tensor("xkvT", [DT, P, NTOK], BF16, kind="ExternalInput")
    # [set(q,kv), {cx,sx,cy,sy}, jtile, p, tok]
    cs_ext = nc.dram_tensor("ropecs", [2, 4, 2, P, NTOK], BF16, kind="ExternalInput")
    # chain-major: [c, p, (dt,128)] so chain c is one contiguous transfer
    wqc_ext = nc.dram_tensor("wqcT", [DT, P, D], BF16, kind="ExternalInput")
    wk_ext = nc.dram_tensor("wkT", [DT, P, D], BF16, kind="ExternalInput")
    wv_ext = nc.dram_tensor("wvT", [DT, P, D], BF16, kind="ExternalInput")
    wo_ext = nc.dram_tensor("woT", [DT, P, D], BF16, kind="ExternalInput")
    qkb_ext = nc.dram_tensor("qkbias", [P, 2 * DT], F32, kind="ExternalInput")
    ones2_ext = nc.dram_tensor("ones2p", [2, P], F32, kind="ExternalInput")
    bpr_ext = nc.dram_tensor("bprime", [P, D], F32, kind="ExternalInput")
    out_ext = nc.dram_tensor("out", [NTOK, D], F32, kind="ExternalOutput")

    with tile.TileContext(nc) as tc:
        with (
            tc.tile_pool(name="const", bufs=1) as cpool,
            tc.tile_pool(name="p_wo", bufs=DT) as p_wo,
            tc.tile_pool(name="p_qt", bufs=DT) as p_qt,
            tc.tile_pool(name="p_ktp", bufs=H) as p_ktp,
            tc.tile_pool(name="p_va", bufs=TT) as p_va,
        ):
            # ---- constants / small inputs ----
            qkbias = cpool.tile([P, 2 * DT], F32)
            nc.sync.dma_start(qkbias[:], qkb_ext.ap())
            bprime = cpool.tile([P, D], F32)  # DMA'd in the late group below
            ones2_f = cpool.tile([2, P], F32)
            ones2 = cpool.tile([2, P], F32R)
            if not USE_GPS_BCAST:
                nc.sync.dma_start(ones2_f[:], ones2_ext.ap())
                nc.vector.tensor_copy(ones2[:], ones2_f[:])
            all1 = cpool.tile([P, H], BF16)
            nc.gpsimd.memset(all1[:], 1.0)
            # warm-up operands (memset first so they're ready before the
            # KTP memsets monopolize gpsimd)
            wu_l = cpool.tile([P, P], BF16)
            wu_r = cpool.tile([P, 512], BF16)
            nc.gpsimd.memset(wu_l[:], 0.0)
            nc.gpsimd.memset(wu_r[:], 0.0)
            # ACT exp-table preload: tiny dummy exp long before attention
            dummy = cpool.tile([1, 1], F32)
            nc.scalar.activation(
                dummy[:], qkbias[0:1, 0:1], mybir.ActivationFunctionType.Exp
            )

            # ---- persistent tiles ----
            WO = [p_wo.tile([P, D], BF16, tag="wo", name=f"wo{i}") for i in range(DT)]
            QT = [p_qt.tile([P, NTOK], BF16, tag="qt", name=f"qt{i}") for i in range(DT)]
            # zero-padded per-head K^T: 64 data rows at partition offset
            # (h%2)*64, other 64 rows stay zero
            KTP = [
                p_ktp.tile([P, NTOK], BF16, tag="ktp", name=f"ktp{i}") for i in range(H)
            ]
            for t in KTP:
                nc.gpsimd.memset(t[:], 0.0)
            VA = [
                p_va.tile([P, H * (HD + 1)], BF16, tag="va", name=f"va{i}")
                for i in range(TT)
            ]

            with (
                tc.tile_pool(name="p_w", bufs=3 * DT) as p_w,
                tc.tile_pool(name="p_xt", bufs=2 * DT) as p_xt,
                tc.tile_pool(name="p_cs", bufs=DT) as p_cs,
                tc.tile_pool(name="p_tmp", bufs=8) as p_tmp,
            ):
                # ---- input DMAs, in dependency-priority order ----
                # sync queue: x + weights; scalar queue: rope cos/sin
                XQ = [
                    p_xt.tile([P, NTOK], BF16, tag="xt", name=f"xq{i}")
                    for i in range(DT)
                ]
                XKV = [
                    p_xt.tile([P, NTOK], BF16, tag="xt", name=f"xkv{i}")
                    for i in range(DT)
                ]
                # cs ring: kv-set DMAs reuse the q-set tiles
                CS = [
                    p_cs.tile([P, NTOK], BF16, tag="cs", name=f"cs{i}")
                    for i in range(DT)
                ]
                WQC = [
                    p_w.tile([P, D], BF16, tag="w", name=f"wqc{i}") for i in range(DT)
                ]
                WK = [p_w.tile([P, D], BF16, tag="w", name=f"wk{i}") for i in range(DT)]
                WV = [p_w.tile([P, D], BF16, tag="w", name=f"wv{i}") for i in range(DT)]
                for i in range(DT):
                    nc.sync.dma_start(XQ[i][:], xq_ext.ap()[i])
                # cos/sin in rope-group consumption order: (cx,sx) j0, j1,
                # then (cy,sy) j0, j1
                for half in range(2):
                    for j in range(2):
                        nc.scalar.dma_start(
                            CS[2 * (2 * half + 0) + j][:], cs_ext.ap()[0, 2 * half, j]
                        )
                        nc.scalar.dma_start(
                            CS[2 * (2 * half + 1) + j][:],
                            cs_ext.ap()[0, 2 * half + 1, j],
                        )
                for c in range(DT):
                    nc.sync.dma_start(WQC[c][:], wqc_ext.ap()[c])
                for i in range(DT):
                    nc.sync.dma_start(XKV[i][:], xkv_ext.ap()[i])

                def rope(xt):
                    for ch in range(2):
                        for g in range(4):
                            half, j = g // 2, g % 2
                            ev = xt[half * 4 + j][:, ch * 512 : (ch + 1) * 512]
                            od = xt[half * 4 + 2 + j][:, ch * 512 : (ch + 1) * 512]
                            sl = slice(ch * 512, (ch + 1) * 512)
                            c_t = CS[2 * (2 * half + 0) + j][:, sl]
                            s_t = CS[2 * (2 * half + 1) + j][:, sl]
                            t1 = p_tmp.tile([P, 512], BF16, tag="tmp", name="t1")
                            t2 = p_tmp.tile([P, 512], BF16, tag="tmp", name="t2")
                            t3 = p_tmp.tile([P, 512], BF16, tag="tmp", name="t3")
                            t4 = p_tmp.tile([P, 512], BF16, tag="tmp", name="t4")
                            nc.vector.tensor_mul(t1[:], ev, c_t)
                            nc.vector.tensor_mul(t2[:], od, s_t)
                            nc.vector.tensor_mul(t3[:], ev, s_t)
                            nc.vector.tensor_mul(t4[:], od, c_t)
                            nc.vector.tensor_sub(ev, t1[:], t2[:])
                            nc.vector.tensor_add(od, t3[:], t4[:])

                # ---- rope q ----
                rope(XQ)
                # late DMA group, emitted after rope-q so the kv cos/sin
                # overwrite of the CS tiles is ordered after rope-q's reads
                for half in range(2):
                    for j in range(2):
                        nc.scalar.dma_start(
                            CS[2 * (2 * half + 0) + j][:], cs_ext.ap()[1, 2 * half, j]
                        )
                        nc.scalar.dma_start(
                            CS[2 * (2 * half + 1) + j][:],
                            cs_ext.ap()[1, 2 * half + 1, j],
                        )
                for i in range(DT):
                    nc.sync.dma_start(WK[i][:], wk_ext.ap()[i])
                for i in range(DT):
                    nc.sync.dma_start(WV[i][:], wv_ext.ap()[i])
                for i in range(DT):
                    nc.sync.dma_start(WO[i][:], wo_ext.ap()[i])
                nc.sync.dma_start(bprime[:], bpr_ext.ap())

                # ---- project Q (chain-major: chain c starts once WQC[c]
                # has landed, overlapping the weight DMA tail) ----
                with tc.tile_pool(name="ps_p", bufs=4, space="PSUM") as ps_p:
                    # p-state warm-up: junk matmuls during the DMA lead-in
                    for _ in range(PE_WARMUP):
                        wu_ps = ps_p.tile([P, 512], F32, tag="p", name="wu")
                        nc.tensor.matmul(
                            wu_ps[:], wu_l[:], wu_r[:], start=True, stop=True
                        )
                    for c in range(DT):
                        for nh in range(2):
                            ps = ps_p.tile([P, 512], F32, tag="p")
                            for dt in range(DT):
                                nc.tensor.matmul(
                                    ps[:],
                                    WQC[c][:, dt * P : (dt + 1) * P],
                                    XQ[dt][:, nh * 512 : (nh + 1) * 512],
                                    start=(dt == 0),
                                    stop=(dt == DT - 1),
                                )
                            nc.scalar.activation(
                                QT[c][:, nh * 512 : (nh + 1) * 512],
                                ps[:],
                                mybir.ActivationFunctionType.Identity,
                                bias=qkbias[:, c : c + 1],
                            )

                    # ---- rope kv (runs on DVE during Q projection) ----
                    rope(XKV)

                    # ---- project K into zero-padded per-head tiles ----
                    for nh in range(2):
                        for c in range(DT):
                            ps = ps_p.tile([P, 512], F32, tag="p")
                            for dt in range(DT):
                                nc.tensor.matmul(
                                    ps[:],
                                    WK[dt][:, c * P : (c + 1) * P],
                                    XKV[dt][:, nh * 512 : (nh + 1) * 512],
                                    start=(dt == 0),
                                    stop=(dt == DT - 1),
                                )
                            qs = slice(nh * 512, (nh + 1) * 512)
                            nc.scalar.activation(
                                KTP[2 * c][0:HD, qs],
                                ps[0:HD, :],
                                mybir.ActivationFunctionType.Identity,
                                bias=qkbias[0:HD, DT + c : DT + c + 1],
                            )
                            nc.scalar.activation(
                                KTP[2 * c + 1][HD:P, qs],
                                ps[HD:P, :],
                                mybir.ActivationFunctionType.Identity,
                                bias=qkbias[HD:P, DT + c : DT + c + 1],
                            )

                    # ---- project V (no bias: folded into bprime) ----
                    for tt in range(TT):
                        nc.vector.tensor_copy(
                            VA[tt][:]
                            .rearrange("p (h c) -> p h c", c=HD + 1)[:, :, HD : HD + 1],
                            all1[:].rearrange("p (h c) -> p h c", c=1),
                        )
                    for tt in range(TT):
                        for nh in range(2):
                            ps = ps_p.tile([P, 512], F32, tag="p")
                            for dt in range(DT):
                                nc.tensor.matmul(
                                    ps[:],
                                    XKV[dt][:, tt * P : (tt + 1) * P],
                                    WV[dt][:, nh * 512 : (nh + 1) * 512],
                                    start=(dt == 0),
                                    stop=(dt == DT - 1),
                                )
                            out_ap = VA[tt][:].rearrange(
                                "p (h c) -> p h c", c=HD + 1
                            )[:, nh * 8 : (nh + 1) * 8, 0:HD]
                            nc.scalar.copy(
                                out_ap, ps[:].rearrange("p (h c) -> p h c", c=HD)
                            )

            # ---- attention (+ first-half output projection in the slots) ----
            with (
                tc.tile_pool(name="p_ot", bufs=DT) as p_ot,
                tc.tile_pool(name="p_a", bufs=12) as p_a,
                tc.tile_pool(name="p_r", bufs=2) as p_r,
                tc.tile_pool(name="p_rbg", bufs=2) as p_rbg,
                tc.tile_pool(name="p_y1", bufs=2 * TT) as p_y1,
                tc.tile_pool(name="p_y", bufs=3) as p_y,
            ):
                OT = [
                    p_ot.tile([P, NTOK], BF16, tag="ot", name=f"ot{i}")
                    for i in range(DT)
                ]
                Y1 = [
                    p_y1.tile([P, 512], BF16, tag="y1", name=f"y1_{i}")
                    for i in range(2 * TT)
                ]

                def make_attention_ops(ps_s, ps_o, ps_rb):
                    def emit_s_exp(h, qh, prev=None):
                        """S + exp for (h, qh); with FINE_INTERLEAVE, the
                        previous block's AV pairs are emitted between this
                        block's S pairs so the PE queue never head-of-line
                        blocks on an exp-gated S matmul."""
                        qt = h // 2
                        a_tiles = []
                        o_prev = None
                        if FINE_INTERLEAVE and prev is not None:
                            (pv_h, pv_qh), pv_a = prev
                            o_prev = ps_o.tile(
                                [HD + 1, 512], F32, tag="oacc", name=f"o{pv_h}_{pv_qh}"
                            )
                        for kp in range(TT // 2):
                            s_ps = ps_s.tile(
                                [P, 1024], F32, tag="s", name=f"s{h}_{qh}_{kp}"
                            )
                            for half in range(2):
                                nc.tensor.matmul(
                                    s_ps[:, half * 512 : (half + 1) * 512],
                                    KTP[h][:, (2 * kp + half) * P : (2 * kp + half + 1) * P],
                                    QT[qt][:, qh * 512 : (qh + 1) * 512],
                                    start=True,
                                    stop=True,
                                )
                            a_t = p_a.tile(
                                [P, 1024], BF16, tag="a", name=f"a{h}_{qh}_{kp}"
                            )
                            nc.scalar.activation(
                                a_t[:], s_ps[:], mybir.ActivationFunctionType.Exp
                            )
                            a_tiles.append(a_t)
                            if o_prev is not None:
                                for kc in (2 * kp, 2 * kp + 1):
                                    nc.tensor.matmul(
                                        o_prev[:],
                                        VA[kc][:, pv_h * (HD + 1) : (pv_h + 1) * (HD + 1)],
                                        pv_a[kc // 2][:, (kc % 2) * 512 : (kc % 2 + 1) * 512],
                                        start=(kc == 0),
                                        stop=(kc == TT - 1),
                                        skip_group_check=True,
                                    )
                        return a_tiles, o_prev

                    def emit_av(h, qh, a_tiles, dnt):
                        o_ps = ps_o.tile(
                            [HD + 1, 512], F32, tag="oacc", name=f"o{h}_{qh}"
                        )
                        for kc in range(TT):
                            nc.tensor.matmul(
                                o_ps[:],
                                VA[kc][:, h * (HD + 1) : (h + 1) * (HD + 1)],
                                a_tiles[kc // 2][:, (kc % 2) * 512 : (kc % 2 + 1) * 512],
                                start=(kc == 0),
                                stop=(kc == TT - 1),
                            )
                        nc.vector.tensor_copy(
                            dnt[0:1, qh * 512 : (qh + 1) * 512], o_ps[HD : HD + 1, :]
                        )
                        return o_ps

                    def emit_norm(h, o0, o1, dnt):
                        r2f = p_r.tile([1, 1024], F32, tag="r2f", name="r2f")
                        nc.vector.reciprocal_approx_fast(r2f[:], dnt[:])
                        if USE_GPS_BCAST:
                            rbg = p_rbg.tile([HD, 1024], F32, tag="rbg", name="rbg")
                            nc.gpsimd.partition_broadcast(rbg[:], r2f[:], channels=HD)
                            rb0, rb1 = rbg[:, 0:512], rbg[:, 512:1024]
                        else:
                            r2s = p_r.tile([2, 512], F32, tag="r2s", name="r2s")
                            nc.sync.dma_start(r2s[:], r2f[:])
                            r2r = p_r.tile([2, 512], F32R, tag="r2r", name="r2r")
                            nc.vector.tensor_copy(r2r[:], r2s[:])
                            rb_ps = ps_rb.tile([P, 512], F32, tag="x", name="rb_ps")
                            nc.tensor.matmul(
                                rb_ps[:], ones2[:], r2r[:], start=True, stop=True
                            )
                            rbg = p_rbg.tile([P, 512], F32, tag="rbg", name="rbg")
                            nc.vector.tensor_copy(rbg[:], rb_ps[:])
                            rb0, rb1 = rbg[0:HD, :], rbg[HD:P, :]
                        qt, po = h // 2, (h % 2) * HD
                        nc.vector.tensor_mul(
                            OT[qt][po : po + HD, 0:512], o0[0:HD, :], rb0
                        )
                        nc.vector.tensor_mul(
                            OT[qt][po : po + HD, 512:1024], o1[0:HD, :], rb1
                        )

                    return emit_s_exp, emit_av, emit_norm

                def emit_y_half1(k):
                    # first half-chain (heads 0-7) for (qc, nh); bias folded in
                    qc, nh = k // 2, k % 2
                    y_ps = ps_y1.tile([P, 512], F32, tag="x", name="y1_ps")
                    for dt in range(DT // 2):
                        nc.tensor.matmul(
                            y_ps[:],
                            OT[dt][:, qc * P : (qc + 1) * P],
                            WO[dt][:, nh * 512 : (nh + 1) * 512],
                            start=(dt == 0),
                            stop=(dt == DT // 2 - 1),
                        )
                    nc.vector.tensor_add(
                        Y1[k][:], y_ps[:], bprime[:, nh * 512 : (nh + 1) * 512]
                    )

                with (
                    tc.tile_pool(name="ps_s", bufs=2, space="PSUM") as ps_s,
                    tc.tile_pool(name="ps_o", bufs=3, space="PSUM") as ps_o,
                    tc.tile_pool(name="ps_x", bufs=1, space="PSUM") as ps_x,
                ):
                    # ps_x serves the rb matmul (fallback) or the half1
                    # out-proj chains (gpsimd-broadcast mode)
                    ps_rb = ps_x
                    ps_y1 = ps_x
                    emit_s_exp, emit_av, emit_norm = make_attention_ops(
                        ps_s, ps_o, ps_rb
                    )

                    blocks = [(h, qh) for h in range(H) for qh in range(2)]
                    prev = None
                    pend = {}  # h -> (o_ps0, dnt)
                    nxt_y1 = 0

                    def finish_prev(o_prev):
                        # denominator stash + pair-norm for the block whose
                        # AV chain just completed
                        ph, pqh = prev[0]
                        if pqh == 0:
                            dnt = p_r.tile([1, 1024], F32, tag="dn", name=f"dn{ph}")
                            nc.vector.tensor_copy(
                                dnt[0:1, 0:512], o_prev[HD : HD + 1, :]
                            )
                            pend[ph] = (o_prev, dnt)
                        else:
                            o0, dnt = pend.pop(ph)
                            nc.vector.tensor_copy(
                                dnt[0:1, 512:1024], o_prev[HD : HD + 1, :]
                            )
                            emit_norm(ph, o0, o_prev, dnt)

                    for b, blk in enumerate(blocks):
                        a_tiles, o_prev = emit_s_exp(*blk, prev=prev)
                        if prev is not None:
                            if FINE_INTERLEAVE:
                                finish_prev(o_prev)
                            else:
                                ph, pqh = prev[0]
                                if pqh == 0:
                                    dnt = p_r.tile(
                                        [1, 1024], F32, tag="dn", name=f"dn{ph}"
                                    )
                                    o0 = emit_av(ph, 0, prev[1], dnt)
                                    pend[ph] = (o0, dnt)
                                else:
                                    o0, dnt = pend.pop(ph)
                                    o1 = emit_av(ph, 1, prev[1], dnt)
                                    emit_norm(ph, o0, o1, dnt)
                        if USE_SPLIT_Y and b >= H and nxt_y1 < 2 * TT:
                            emit_y_half1(nxt_y1)
                            nxt_y1 += 1
                        prev = (blk, a_tiles)
                    # last block's AV + norm
                    ph, pqh = prev[0]
                    o0, dnt = pend.pop(ph)
                    o1 = emit_av(ph, 1, prev[1], dnt)
                    emit_norm(ph, o0, o1, dnt)
                    while USE_SPLIT_Y and nxt_y1 < 2 * TT:
                        emit_y_half1(nxt_y1)
                        nxt_y1 += 1

                # ---- output projection: second half (heads 8-15) ----
                with tc.tile_pool(name="ps_y", bufs=4, space="PSUM") as ps_y:
                    lo = DT // 2 if USE_SPLIT_Y else 0
                    for qc in range(TT):
                        y_t = p_y.tile([P, D], F32, tag="y")
                        for nh in range(2):
                            y_ps = ps_y.tile([P, 512], F32, tag="y")
                            for dt in range(lo, DT):
                                nc.tensor.matmul(
                                    y_ps[:],
                                    OT[dt][:, qc * P : (qc + 1) * P],
                                    WO[dt][:, nh * 512 : (nh + 1) * 512],
                                    start=(dt == lo),
                                    stop=(dt == DT - 1),
                                )
                            if USE_SPLIT_Y:
                                nc.vector.tensor_add(
                                    y_t[:, nh * 512 : (nh + 1) * 512],
                                    y_ps[:],
                                    Y1[2 * qc + nh][:],
                                )
                            else:
                                nc.vector.tensor_add(
                                    y_t[:, nh * 512 : (nh + 1) * 512],
                                    y_ps[:],
                                    bprime[:, nh * 512 : (nh + 1) * 512],
                                )
                        nc.sync.dma_start(out_ext.ap()[qc * P : (qc + 1) * P, :], y_t[:])

    nc.compile()
    return nc


# ---------------------------------------------------------------------------
# host side
# ---------------------------------------------------------------------------

def host_prep(x, coords, is_context, rope_cache,
              ctx_in_w, ctx_in_b, ctx_out_w, ctx_out_b,
              tgt_in_w, tgt_in_b, tgt_out_w, tgt_out_b):
    """Compute per-core input maps + the scatter indices."""
    x = np.asarray(x, np.float32)
    coords = np.asarray(coords, np.float32)
    is_context = np.asarray(is_context, bool)
    rope_cache = np.asarray(rope_cache, np.float32)

    keys = np.where(is_context, 0, 1).astype(np.int32)
    order = np.argsort(keys, axis=1, kind="stable")
    ctx_idx = order[:, :NCTX]
    tgt_idx = order[:, NCTX:]

    # rope positions (mirror reference fp32 arithmetic)
    cn = np.clip(
        coords / np.float32(IMAGE_SIZE) * np.float32(MAX_POS - 1), 0, MAX_POS - 1
    )
    y_pos = cn[..., 0].astype(np.int32)
    x_pos = cn[..., 1].astype(np.int32)
    # [B, K, 256] each
    cs_all = (
        rope_cache[x_pos, :, 0],
        rope_cache[x_pos, :, 1],
        rope_cache[y_pos, :, 0],
        rope_cache[y_pos, :, 1],
    )

    def w_pack(in_w, in_b, out_w, out_b):
        w = np.array(in_w, np.float32)
        bvec = np.array(in_b, np.float32)
        w[0:D] *= np.float32(0.125)
        bvec = bvec.copy()
        bvec[0:D] *= np.float32(0.125)
        wT = np.ascontiguousarray(w.T)[PERM]  # [din(perm), 3*D]
        proj = []
        for which in range(3):
            proj.append(
                np.ascontiguousarray(
                    wT[:, which * D : (which + 1) * D].reshape(DT, P, D)
                ).astype(NPBF16)
            )
        # chain-major Q weights: [c, p, dt*128+c2]
        wqc = np.ascontiguousarray(
            proj[0].reshape(DT, P, DT, P).transpose(2, 1, 0, 3).reshape(DT, P, D)
        )
        qkbias = np.ascontiguousarray(
            bvec[0 : 2 * D].reshape(2 * DT, P).T
        ).astype(np.float32)
        out_w = np.asarray(out_w, np.float32)
        woT = (
            np.ascontiguousarray(out_w.T.reshape(DT, P, D)).astype(NPBF16)
        )
        bprime = out_b.astype(np.float32) + out_w @ bvec[2 * D : 3 * D]
        bprime = np.ascontiguousarray(
            np.broadcast_to(bprime[None, :], (P, D))
        ).astype(np.float32)
        return wqc, proj[1], proj[2], woT, qkbias, bprime

    packs = [w_pack(ctx_in_w, ctx_in_b, ctx_out_w, ctx_out_b),
             w_pack(tgt_in_w, tgt_in_b, tgt_out_w, tgt_out_b)]

    def cs_pack(b, idx):
        # [4, 2, P, NTOK]
        out = np.empty((4, 2, P, NTOK), NPBF16)
        for i, arr in enumerate(cs_all):
            t = arr[b][idx].T  # [256, NTOK]
            out[i] = t.reshape(2, P, NTOK).astype(NPBF16)
        return out

    def xt_pack(b, idx):
        t = np.ascontiguousarray(x[b][idx][:, PERM].T)  # [D(perm), NTOK]
        return t.reshape(DT, P, NTOK).astype(NPBF16)

    in_maps = []
    scatter = []
    for c in range(8):
        b, role = c // 2, c % 2
        q_idx = ctx_idx[b] if role == 0 else tgt_idx[b]
        kv_idx = ctx_idx[b]
        ropecs = np.stack([cs_pack(b, q_idx), cs_pack(b, kv_idx)])
        wqc, wkT, wvT, woT, qkbias, bprime = packs[role]
        ones2p = np.zeros((2, P), np.float32)
        ones2p[0, 0:HD] = 1.0
        ones2p[1, HD:P] = 1.0
        in_maps.append({
            "ones2p": ones2p,
            "xqT": xt_pack(b, q_idx),
            "xkvT": xt_pack(b, kv_idx),
            "ropecs": np.ascontiguousarray(ropecs),
            "wqcT": wqc,
            "wkT": wkT,
            "wvT": wvT,
            "woT": woT,
            "qkbias": qkbias,
            "bprime": bprime,
        })
        scatter.append((b, q_idx))
    return in_maps, scatter


_NC_CACHE = None


def kernel(**inputs):
    global _NC_CACHE
    in_maps, scatter = host_prep(**inputs)
    if _NC_CACHE is None:
        _NC_CACHE = build_nc()
    nc = _NC_CACHE
    res = run_bass_kernel_spmd(nc, in_maps, core_ids=list(range(8)))
    x = np.asarray(inputs["x"], np.float32)
    out = np.zeros_like(x)
    for c in range(8):
        b, q_idx = scatter[c]
        out[b][q_idx] = res.results[c]["out"]
    return out
